# revision 18
# baseline (speedup 1.0000x reference)
"""Trainium2 Bass kernel for nn_ConvolutionalNMPBlock.

Self-contained: takes full (unsharded) inputs, shards batch across 8
NeuronCores (2 elements each), runs a fused Bass/Tile kernel, gathers.

v4: fp8 DoubleRow matmuls for conv1/conv2/A-acc/GRU-hh/GRU-ih-conv
(2x PE rate), f32r for the GRU-ih xnm half (kills the dominant bf16
quantization error, buying the budget the fp8 paths spend).
Schedule: per-quarter fine-grained front (DMA -> split -> transpose ->
se/conv1/msg as each quarter lands), then wave-interleaved main phase
(G/exp(c) [ACT] || conv2(c) [PE] || acc+GRU(c-1)); the next element's
front+factors are emitted inside the previous element's waves.
"""
import numpy as np
import ml_dtypes

BS, N, D = 16, 2048, 256
NCORE = 8
PER = BS // NCORE          # batch elements per core
EPS = 1e-5
NB = N // 128              # 16 row blocks
LC = N // 512              # 4 column chunks of 512
DC = D // 128              # 2 channel blocks
KT = 17                    # conv2 taps
BF = ml_dtypes.bfloat16
E4 = ml_dtypes.float8_e4m3

_built = {}                # cfg key -> compiled nc

# staged toggles (all on = target config)
CFG = dict(fp8_conv=True, fp8_acc=True, fp8_hh=True, fp8_ih=True,
           f32r_ih=True, se_chains=1)


def _build(use_mask: bool, use_bias: bool = True, loop_n: int = 1,
           skip: frozenset = frozenset(), body_reps: int = 1, cfg=None):
    from concourse import bacc, tile
    import concourse.mybir as mybir
    from contextlib import ExitStack

    cfg = dict(CFG if cfg is None else cfg)
    FP8C = cfg["fp8_conv"]
    FP8A = cfg["fp8_acc"]
    FP8H = cfg["fp8_hh"]
    FP8I = cfg["fp8_ih"]
    F32R = cfg["f32r_ih"]
    SE_N = cfg["se_chains"]

    f32 = mybir.dt.float32
    f32r = mybir.dt.float32r
    bf16 = mybir.dt.bfloat16
    e4 = mybir.dt.float8e4
    cdt = e4 if FP8C else bf16        # conv1/conv2 operand dtype
    adt = e4 if FP8A else bf16        # at/msg dtype
    hdt = e4 if FP8H else bf16        # hh operand dtype
    idt = e4 if FP8I else bf16        # xcv / wih_c dtype
    ndt = f32r if F32R else bf16      # xnm / wih_n dtype
    AF = mybir.ActivationFunctionType
    OP = mybir.AluOpType
    PM = mybir.MatmulPerfMode
    DR = PM.DoubleRow

    nc = bacc.Bacc("TRN2", target_bir_lowering=False, debug=False,
                   num_devices=NCORE)

    def din(name, shape, dt=f32):
        return nc.dram_tensor(name, shape, dt, kind="ExternalInput").ap()

    x_d = din("x", (PER, N, D))
    w1_d = din("w1t", (128, DC, DC, 128), cdt)         # [p=kin, kc, mc, m]
    w2_d = din("w2t", (128, DC, KT, DC, 128), cdt)     # [p, kc, tap, mc, m]
    sh1_d = din("sh1", (128, DC))
    sh2_d = din("sh2", (128, DC))
    wmsg_d = din("wmsgt", (128, DC, D), bf16)          # [p, kc, f]
    bmsg_d = din("bmsg", (1, D), bf16)
    wseh_d = din("wseth", (128, DC, 3), bf16)
    wsel_d = din("wsetl", (128, DC, 3), bf16)
    bse_d = din("bse", (1, 3), bf16)
    wihc_d = din("wihct", (128, DC, 3 * D), idt)       # [p, kc, f] xcv half
    wihn_d = din("wihnt", (128, DC, 3 * D), ndt)       # [p, kc, f] xnm half
    whh_d = din("whht", (128, DC, 3 * D), hdt)
    brz_d = din("brow_rz", (1, 2 * D), bf16)
    bgin_d = din("brow_gin", (1, D), bf16)
    bghn_d = din("brow_ghn", (1, D), bf16)
    ones_d = din("ones128", (1, 128), bf16)
    ones5_d = din("ones512", (1, 512), bf16)
    ones2n_d = din("ones2n", (2, N), bf16)
    if use_mask:
        mt_d = din("maskt", (PER, N, N), bf16)
    out_d = nc.dram_tensor("out", (PER, N, D), f32, kind="ExternalOutput").ap()

    with tile.TileContext(nc) as tc, ExitStack() as ctx:
        if loop_n > 1:
            ctx.enter_context(tc.For_i(0, loop_n, 1))
        wp = ctx.enter_context(tc.tile_pool(name="wp", bufs=1))
        big = ctx.enter_context(tc.tile_pool(name="big", bufs=1))
        rawp = ctx.enter_context(tc.tile_pool(name="rawp", bufs=1))
        atp = ctx.enter_context(tc.tile_pool(name="atp", bufs=12))
        cvp = ctx.enter_context(tc.tile_pool(name="cvp", bufs=2))
        gtp = ctx.enter_context(tc.tile_pool(name="gtp", bufs=1))
        ps = ctx.enter_context(tc.tile_pool(name="ps", bufs=2, space="PSUM"))
        pa = ctx.enter_context(tc.tile_pool(name="pa", bufs=1, space="PSUM"))
        gpp = ctx.enter_context(tc.tile_pool(name="gpp", bufs=2, space="PSUM"))

        # ---- load weights (persistent) ----
        w1 = wp.tile([128, DC, DC, 128], cdt, tag="w1")
        nc.gpsimd.dma_start(w1[:], w1_d[:])
        sh1 = wp.tile([128, DC], f32, tag="sh1")
        nc.gpsimd.dma_start(sh1[:], sh1_d[:])
        sh2 = wp.tile([128, DC], f32, tag="sh2")
        nc.gpsimd.dma_start(sh2[:], sh2_d[:])
        bmsg = wp.tile([1, D], bf16, tag="bmsg")
        nc.gpsimd.dma_start(bmsg[:], bmsg_d[:])
        wseh = wp.tile([128, DC, 3], bf16, tag="wseh")
        nc.gpsimd.dma_start(wseh[:], wseh_d[:])
        wsel = wp.tile([128, DC, 3], bf16, tag="wsel")
        nc.gpsimd.dma_start(wsel[:], wsel_d[:])
        bse = wp.tile([1, 3], bf16, tag="bse")
        nc.gpsimd.dma_start(bse[:], bse_d[:])
        ones = wp.tile([1, 128], bf16, tag="ones")
        nc.gpsimd.dma_start(ones[:], ones_d[:])
        ones5 = wp.tile([1, 512], bf16, tag="ones5")
        nc.gpsimd.dma_start(ones5[:], ones5_d[:])
        w2 = wp.tile([128, DC, KT, DC, 128], cdt, tag="w2")
        nc.sync.dma_start(w2[:], w2_d[:])
        wmsg = wp.tile([128, DC, D], bf16, tag="wmsg")
        nc.sync.dma_start(wmsg[:], wmsg_d[:])
        wihc = wp.tile([128, DC, 3 * D], idt, tag="wihc")
        nc.sync.dma_start(wihc[:], wihc_d[:])
        wihn = wp.tile([128, DC, 3 * D], ndt, tag="wihn")
        nc.sync.dma_start(wihn[:], wihn_d[:])
        whh = wp.tile([128, DC, 3 * D], hdt, tag="whh")
        nc.sync.dma_start(whh[:], whh_d[:])
        brz = wp.tile([1, 2 * D], bf16, tag="brz")
        nc.sync.dma_start(brz[:], brz_d[:])
        bgin = wp.tile([1, D], bf16, tag="bgin")
        nc.sync.dma_start(bgin[:], bgin_d[:])
        bghn = wp.tile([1, D], bf16, tag="bghn")
        nc.sync.dma_start(bghn[:], bghn_d[:])
        from concourse.masks import make_identity
        identf = wp.tile([128, 128], f32, tag="identf")
        make_identity(nc, identf[:])
        if "noG" in skip:
            atc = wp.tile([128, 1024], adt, tag="atc")
            nc.vector.memset(atc[:], 0.001)

        qn = NB // 4
        quarters = [slice(h * qn, (h + 1) * qn) for h in range(4)]
        se_chains = (((wseh, 'h'),), ((wseh, 'h'), (wseh, 'l')),
                     ((wseh, 'h'), (wseh, 'l'), (wsel, 'h')))[SE_N - 1]

        def front(el):
            """Per-quarter: x load, split, transposes, e4 copy, then the
            quarter's se / conv1 / msg work as soon as its transpose lands."""
            t = {}
            xnf = rawp.tile([128, NB, D], f32, tag="xnf", bufs=1)
            xnh = rawp.tile([128, NB, D], bf16, tag="xnh", bufs=2)
            xnl = rawp.tile([128, NB, D], bf16, tag="xnl", bufs=1)
            # layout: xth[p_c, nb, dc, p_n] == xT[dc*128+p_c, nb*128+p_n]
            xth = big.tile([128, NB, DC, 128], bf16, tag="xth", bufs=2)
            xtl = big.tile([128, NB, DC, 128], bf16, tag="xtl", bufs=2)
            xt8 = None
            if FP8C or FP8H:
                xt8 = big.tile([128, DC, NB, 128], e4, tag="xt8", bufs=2)
            h1 = big.tile([128, DC, N + 16], cdt, tag="h1", bufs=2)
            msg = big.tile([128, NB, D], adt, tag="msg", bufs=2)
            xcv = big.tile([128, DC, N], idt, tag="xcv", bufs=2)
            xnm = big.tile([128, DC, N], ndt, tag="xnm", bufs=1)
            st = big.tile([3, N], f32, tag="st", bufs=1)
            sa = big.tile([128, N], bf16, tag="sa", bufs=2)
            sb = big.tile([128, N], bf16, tag="sb", bufs=2)
            # wide factor work tiles (per-quarter slices fill them)
            stw = cvp.tile([128, NB, 3], f32, tag="stw", bufs=2)
            ssqw = cvp.tile([128, NB, 3], f32, tag="ssqw", bufs=2)
            sqw = cvp.tile([128, NB, 1], f32, tag="sqw", bufs=2)
            h2s = cvp.tile([128, NB, 3], bf16, tag="h2s", bufs=2)
            hsw = cvp.tile([128, NB, 3], bf16, tag="hsw", bufs=2)
            hq = cvp.tile([128, NB, 1], bf16, tag="hq", bufs=2)
            saw = cvp.tile([128, NB, 13], f32, tag="saw", bufs=2)
            sbw = cvp.tile([128, NB, 13], f32, tag="sbw", bufs=2)
            t.update(xnh=xnh, xnl=xnl, xth=xth, xtl=xtl, xt8=xt8, h1=h1,
                     msg=msg, xcv=xcv, xnm=xnm, st=st, sa=sa, sb=sb,
                     at_store={})
            xt_of = {'h': xth, 'l': xtl}
            if "wide" in skip:
                nc.vector.memset(sa[:], 0.01)
                nc.vector.memset(sb[:], 0.01)
            if "conv1" in skip:
                nc.vector.memset(h1[:], 0.001)
            nc.vector.memset(h1[:, :, 0:8], 0.0)
            nc.vector.memset(h1[:, :, N + 8:N + 16], 0.0)
            if "msg" in skip:
                nc.vector.memset(msg[:], 0.001)
            if "conv2" in skip:
                nc.vector.memset(xcv[:], 0.0)
            if "A" in skip:
                nc.vector.memset(xnm[:], 0.0)
            if "s" in skip:
                nc.vector.memset(st[:], 0.01)

            for c, hs in enumerate(quarters):
                nc.sync.dma_start(
                    xnf[:, hs, :],
                    x_d[el, c * (N // 4):(c + 1) * (N // 4), :].rearrange(
                        "(nb p) d -> p nb d", p=128))
                if "split" not in skip:
                    nc.vector.tensor_copy(xnh[:, hs, :], xnf[:, hs, :])
                    nc.vector.scalar_tensor_tensor(xnl[:, hs, :],
                                                   xnf[:, hs, :],
                                                   1.0, xnh[:, hs, :],
                                                   OP.mult, OP.subtract)
                if "transp" not in skip:
                    nc.sync.dma_start_transpose(
                        xth[:, hs].rearrange("p nb dc pn -> p (nb dc) pn"),
                        xnh[:, hs, :])
                    nc.sync.dma_start_transpose(
                        xtl[:, hs].rearrange("p nb dc pn -> p (nb dc) pn"),
                        xnl[:, hs, :])
                    if xt8 is not None:
                        # e4 channel-major, kc-major layout for DoubleRow:
                        # xt8[p, kc, nb, m] == xT[kc*128+p, nb*128+m]
                        for kc in range(DC):
                            nc.gpsimd.tensor_copy(xt8[:, kc, hs, :],
                                                  xth[:, hs, kc, :])
                # ---- s chunk: s = x @ w_se.T (+ b_se) ----
                if "s" not in skip:
                    pss = ps.tile([3, 512], f32, tag="ps")
                    first = True
                    for kc in range(DC):
                        for wi_, (wse_, which) in enumerate(se_chains):
                            last = (not use_bias) and kc == DC - 1 and \
                                wi_ == len(se_chains) - 1
                            nc.tensor.matmul(
                                pss[:], wse_[:, kc, :],
                                xt_of[which][:, 4 * c:4 * (c + 1), kc, :],
                                start=first, stop=last,
                                skip_group_check=True)
                            first = False
                    if use_bias:
                        nc.tensor.matmul(pss[:], bse[:], ones5[:], start=False,
                                         stop=True, skip_group_check=True)
                    nc.vector.tensor_copy(st[:, c * 512:(c + 1) * 512], pss[:])
                # ---- conv1 chunk (1x1) + bn1 + relu -> h1 ----
                for mc in range(0 if "conv1" in skip else DC):
                    pc = ps.tile([128, 512], f32, tag="ps")
                    if FP8C:
                        nc.tensor.matmul(
                            pc[:], w1[:, :, mc, :],
                            xt8[:, :, 4 * c:4 * (c + 1), :].rearrange(
                                "p kc nb m -> p kc (nb m)"),
                            start=True, stop=True, perf_mode=DR,
                            skip_group_check=True)
                    else:
                        for kc in range(DC):
                            nc.tensor.matmul(pc[:], w1[:, kc, mc, :],
                                             xth[:, 4 * c:4 * (c + 1), kc, :],
                                             start=(kc == 0),
                                             stop=(kc == DC - 1),
                                             skip_group_check=True)
                    nc.scalar.activation(
                        h1[:, mc, 8 + c * 512:8 + (c + 1) * 512],
                        pc[:], AF.Relu, bias=sh1[:, mc:mc + 1])
                # ---- msg chunk = relu(x @ w_msg.T + b_msg), n-major ----
                # two n-blocks share one psum tile (one 512-wide relu each)
                for nbp in ((2 * c, 2 * c + 1) if "msg" not in skip else ()):
                    pm = ps.tile([128, 512], f32, tag="ps")
                    for hh_ in range(2):
                        nb = 2 * nbp + hh_
                        hsl = slice(hh_ * D, (hh_ + 1) * D)
                        for kc in range(DC):
                            nc.tensor.matmul(pm[:, hsl], xth[:, nb, kc, :],
                                             wmsg[:, kc, :], start=(kc == 0),
                                             stop=(not use_bias and
                                                   kc == DC - 1),
                                             skip_group_check=True)
                        if use_bias:
                            nc.tensor.matmul(pm[:, hsl], ones[:], bmsg[:],
                                             start=False, stop=True,
                                             skip_group_check=True)
                    nc.scalar.activation(msg[:, 2 * nbp:2 * nbp + 2, :],
                                         pm[:], AF.Relu)
                # ---- factor quarter: build 13-row hi/lo stacks sa/sb ----
                # k-row pairs (SA | SB): 0:3 (2s_hi | s_hi), 3:6 (2s_lo |
                # s_hi), 6:9 (2s_hi | s_lo), 9,10 (1 | -sq_hi, -sq_lo),
                # 11,12 (-sq_hi, -sq_lo | 1), built at 128-lane width,
                # PE-transposed to the narrow 13-row layout.
                # sa: rows for j-block g at partitions 32*(g%4):+13, cols
                # g*128:+128.  sb: rows replicated at all 4 partition bases.
                if "s" in skip or "wide" in skip:
                    continue
                gsl = slice(4 * c, 4 * (c + 1))
                # narrow->wide PE transposes: stw[p, g, :] == st[:, g*128+p]
                ptt = ps.tile([128, 512], f32, tag="ps")
                for j in range(4):
                    g = 4 * c + j
                    nc.tensor.transpose(ptt[:, j * 4:j * 4 + 3],
                                        st[:, g * 128:(g + 1) * 128],
                                        identf[0:3, 0:3])
                nc.vector.tensor_copy(
                    stw[:, gsl, :],
                    ptt[:, 0:16].rearrange("p (g k) -> p g k", k=4)[:, :, 0:3])
                nc.scalar.activation(ssqw[:, gsl, :], stw[:, gsl, :],
                                     AF.Square)
                nc.vector.tensor_tensor(sqw[:, gsl, :], ssqw[:, gsl, 0:1],
                                        ssqw[:, gsl, 1:2], OP.add)
                nc.vector.tensor_tensor(sqw[:, gsl, :], sqw[:, gsl, :],
                                        ssqw[:, gsl, 2:3], OP.add)
                nc.vector.tensor_scalar(h2s[:, gsl, :], stw[:, gsl, :], 2.0,
                                        None, OP.mult)
                nc.vector.tensor_copy(hsw[:, gsl, :], stw[:, gsl, :])
                nc.vector.tensor_scalar(hq[:, gsl, :], sqw[:, gsl, :], -1.0,
                                        None, OP.mult)
                nc.vector.tensor_copy(saw[:, gsl, 0:3], h2s[:, gsl, :])
                nc.vector.scalar_tensor_tensor(saw[:, gsl, 3:6],
                                               stw[:, gsl, :], 2.0,
                                               saw[:, gsl, 0:3], OP.mult,
                                               OP.subtract)
                nc.vector.tensor_copy(saw[:, gsl, 6:9], saw[:, gsl, 0:3])
                nc.vector.memset(saw[:, gsl, 9:11], 1.0)
                nc.vector.tensor_copy(saw[:, gsl, 11:12], hq[:, gsl, :])
                nc.vector.scalar_tensor_tensor(saw[:, gsl, 12:13],
                                               sqw[:, gsl, :],
                                               -1.0, saw[:, gsl, 11:12],
                                               OP.mult, OP.subtract)
                nc.vector.tensor_copy(sbw[:, gsl, 0:3], hsw[:, gsl, :])
                nc.vector.tensor_copy(sbw[:, gsl, 3:6], sbw[:, gsl, 0:3])
                nc.vector.scalar_tensor_tensor(sbw[:, gsl, 6:9],
                                               stw[:, gsl, :], 1.0,
                                               sbw[:, gsl, 0:3], OP.mult,
                                               OP.subtract)
                nc.vector.tensor_copy(sbw[:, gsl, 9:11], saw[:, gsl, 11:13])
                nc.vector.memset(sbw[:, gsl, 11:13], 1.0)
                # wide->narrow via PE transposes (f32), bf16 cast on copy
                ptn = ps.tile([128, 512], f32, tag="ps")
                for j in range(4):
                    g = 4 * c + j
                    nc.tensor.transpose(ptn[0:13, j * 128:(j + 1) * 128],
                                        saw[:, g, :], identf[:])
                for j in range(4):
                    nc.vector.tensor_copy(
                        sa[32 * j:32 * j + 13,
                           (4 * c + j) * 128:(4 * c + j + 1) * 128],
                        ptn[0:13, j * 128:(j + 1) * 128])
                ptn2 = ps.tile([128, 512], f32, tag="ps", name="ptn2")
                for j in range(4):
                    g = 4 * c + j
                    nc.tensor.transpose(ptn2[0:13, j * 128:(j + 1) * 128],
                                        sbw[:, g, :], identf[:])
                cs = slice(c * 512, (c + 1) * 512)
                nc.vector.tensor_copy(sb[0:13, cs], ptn2[0:13, :])
                nc.vector.tensor_copy(sb[32:45, cs], ptn2[0:13, :])
                nc.gpsimd.tensor_copy(sb[64:77, cs], sb[0:13, cs])
                nc.gpsimd.tensor_copy(sb[96:109, cs], sb[0:13, cs])
            return t

        def g_exp(t, el, ic):
            # G matmuls in PAIRS into a 2-bank psum tile: halves the exp
            # op count (the ~185ns ACT fixed overhead per op)
            sa, sb = t["sa"], t["sb"]
            for jp in range(NB // 2):
                if "noG" in skip:
                    t["at_store"][(ic, jp)] = atc
                    continue
                pgm = gpp.tile([128, 1024], f32, tag="gp")
                for h in range(2):
                    jb = 2 * jp + h
                    rg = 32 * (jb % 4)
                    nc.tensor.matmul(pgm[:, h * 512:(h + 1) * 512],
                                     sa[rg:rg + 13, jb * 128:(jb + 1) * 128],
                                     sb[rg:rg + 13, ic * 512:(ic + 1) * 512],
                                     start=True, stop=True,
                                     skip_group_check=True,
                                     tile_position=(rg, 0))
                at = atp.tile([128, 1024], adt, tag="at")
                if "exp2dve" in skip:
                    nc.vector.tensor_copy(at[:], pgm[:])
                else:
                    nc.scalar.activation(at[:], pgm[:], AF.Exp)
                if use_mask:
                    mtt = cvp.tile([128, 1024], bf16, tag="mtt")
                    for h in range(2):
                        jb = 2 * jp + h
                        nc.sync.dma_start(
                            mtt[:, h * 512:(h + 1) * 512],
                            mt_d[el, jb * 128:(jb + 1) * 128,
                                 ic * 512:(ic + 1) * 512])
                    nc.vector.tensor_tensor(at[:], at[:], mtt[:], OP.mult)
                t["at_store"][(ic, jp)] = at

        def conv2_chunk(t, mc, c):
            # ---- conv2 (17 taps) + bn2 + residual + relu -> x_convT ----
            h1, xth, xtl, xcv = t["h1"], t["xth"], t["xtl"], t["xcv"]
            pc2 = ps.tile([128, 512], f32, tag="ps")
            if FP8C:
                for tp in range(KT):
                    nc.tensor.matmul(
                        pc2[:], w2[:, :, tp, mc, :],
                        h1[:, :, c * 512 + tp:c * 512 + tp + 512],
                        start=(tp == 0), stop=(tp == KT - 1),
                        perf_mode=DR, skip_group_check=True)
            else:
                first = True
                for kc in range(DC):
                    for tp in range(KT):
                        nc.tensor.matmul(
                            pc2[:], w2[:, kc, tp, mc, :],
                            h1[:, kc, c * 512 + tp:c * 512 + tp + 512],
                            start=first,
                            stop=(kc == DC - 1 and tp == KT - 1),
                            skip_group_check=True)
                        first = False
            tv = cvp.tile([128, 4, 128], f32, tag="cv", bufs=2)
            nc.vector.tensor_tensor(tv[:], pc2[:].rearrange(
                "p (a b) -> p a b", b=128),
                xth[:, 4 * c:4 * (c + 1), mc, :], OP.add)
            nc.vector.tensor_tensor(tv[:], tv[:],
                                    xtl[:, 4 * c:4 * (c + 1), mc, :],
                                    OP.add)
            nc.scalar.activation(
                xcv[:, mc, c * 512:(c + 1) * 512],
                tv[:].rearrange("p a b -> p (a b)"),
                AF.Relu, bias=sh2[:, mc:mc + 1])

        def emit_acc(t, ic):
            # x_nmpT[d, i] = sum_j exp(-dist[j,i]) * msg[j, d]
            msg, xnm, at_store = t["msg"], t["xnm"], t["at_store"]
            accs = [pa.tile([128, 512], f32, tag="acc0", name="a0"),
                    pa.tile([128, 512], f32, tag="acc1", name="a1")]
            for mc in range(DC):
                if FP8A:
                    for jp in range(NB // 2):
                        at_t = at_store[(ic, jp)]
                        nc.tensor.matmul(
                            accs[mc][:],
                            msg[:, 2 * jp:2 * jp + 2,
                                mc * 128:(mc + 1) * 128],
                            at_t[:].rearrange("p (k c) -> p k c", k=2),
                            start=(jp == 0), stop=(jp == NB // 2 - 1),
                            perf_mode=DR, skip_group_check=True)
                else:
                    for jb in range(NB):
                        nc.tensor.matmul(
                            accs[mc][:],
                            msg[:, jb, mc * 128:(mc + 1) * 128],
                            at_store[(ic, jb // 2)][
                                :, (jb % 2) * 512:(jb % 2 + 1) * 512],
                            start=(jb == 0), stop=(jb == NB - 1),
                            skip_group_check=True)
            for mc in range(DC):
                nc.vector.tensor_copy(xnm[:, mc, ic * 512:(ic + 1) * 512],
                                      accs[mc][:])

        def gru_mm(t, nb):
            """GRU gate matmuls for one n-block into ONE 1024-col psum tile:
            [rz 0:512 | gi_n 512:768 | gh_n 768:1024]."""
            xcv, xnm, xth, xt8 = (t["xcv"], t["xnm"], t["xth"], t["xt8"])
            sl = slice(nb * 128, (nb + 1) * 128)
            pg = gpp.tile([128, 1024], f32, tag="gp")
            # --- rz (bank 0) ---
            if FP8I:
                nc.tensor.matmul(pg[:, 0:512], xcv[:, :, sl], wihc[:, :, 0:512],
                                 start=True, stop=False, perf_mode=DR,
                                 skip_group_check=True)
            else:
                for kc in range(DC):
                    nc.tensor.matmul(pg[:, 0:512], xcv[:, kc, sl],
                                     wihc[:, kc, 0:512],
                                     start=(kc == 0), stop=False,
                                     skip_group_check=True)
            for kc in range(DC):
                nc.tensor.matmul(pg[:, 0:512], xnm[:, kc, sl],
                                 wihn[:, kc, 0:512],
                                 start=False, stop=False,
                                 skip_group_check=True)
            if FP8H:
                nc.tensor.matmul(pg[:, 0:512], xt8[:, :, nb, :],
                                 whh[:, :, 0:512],
                                 start=False, stop=(not use_bias),
                                 perf_mode=DR, skip_group_check=True)
            else:
                for kc in range(DC):
                    nc.tensor.matmul(pg[:, 0:512], xth[:, nb, kc, :],
                                     whh[:, kc, 0:512],
                                     start=False,
                                     stop=(not use_bias and kc == DC - 1),
                                     skip_group_check=True)
            if use_bias:
                nc.tensor.matmul(pg[:, 0:512], ones[:], brz[:], start=False,
                                 stop=True, skip_group_check=True)
            # --- gi_n (bank 1 lo) ---
            if FP8I:
                nc.tensor.matmul(pg[:, 512:768], xcv[:, :, sl],
                                 wihc[:, :, 512:768],
                                 start=True, stop=False, perf_mode=DR,
                                 skip_group_check=True)
            else:
                for kc in range(DC):
                    nc.tensor.matmul(pg[:, 512:768], xcv[:, kc, sl],
                                     wihc[:, kc, 512:768],
                                     start=(kc == 0), stop=False,
                                     skip_group_check=True)
            for kc in range(DC):
                nc.tensor.matmul(pg[:, 512:768], xnm[:, kc, sl],
                                 wihn[:, kc, 512:768],
                                 start=False,
                                 stop=(not use_bias and kc == DC - 1),
                                 skip_group_check=True)
            if use_bias:
                nc.tensor.matmul(pg[:, 512:768], ones[:], bgin[:], start=False,
                                 stop=True, skip_group_check=True)
            # --- gh_n (bank 1 hi) ---
            if FP8H:
                nc.tensor.matmul(pg[:, 768:1024], xt8[:, :, nb, :],
                                 whh[:, :, 512:768],
                                 start=True, stop=(not use_bias),
                                 perf_mode=DR, skip_group_check=True)
            else:
                for kc in range(DC):
                    nc.tensor.matmul(pg[:, 768:1024], xth[:, nb, kc, :],
                                     whh[:, kc, 512:768], start=(kc == 0),
                                     stop=(not use_bias and kc == DC - 1),
                                     skip_group_check=True)
            if use_bias:
                nc.tensor.matmul(pg[:, 768:1024], ones[:], bghn[:],
                                 start=False, stop=True,
                                 skip_group_check=True)
            return pg

        def gru_pair(t, el, nba):
            """Two GRU blocks, stage-sorted emission so neither engine's
            in-order queue blocks on a cross-engine dependency."""
            xnh = t["xnh"]
            nbs = (nba, nba + 1)
            pgs = [gru_mm(t, nb) for nb in nbs]
            # trz = tanh(0.5*[gi_r+gh_r | gi_z+gh_z]);
            # sigmoid(x) = 0.5*tanh(x/2)+0.5 folded into the stt chains.
            trzs, qs = [], []
            for pg in pgs:
                trz = gtp.tile([128, 2 * D], f32, tag="trz", bufs=3)
                nc.scalar.activation(trz[:], pg[:, 0:512], AF.Tanh, scale=0.5)
                trzs.append(trz)
            for pg, trz in zip(pgs, trzs):
                # q = r*gh_n + gi_n = 0.5*(tr+1)*gh_n + gi_n
                q = gtp.tile([128, D], f32, tag="q", bufs=3)
                nc.vector.scalar_tensor_tensor(q[:], trz[:, 0:D], 1.0,
                                               pg[:, 768:1024], OP.add,
                                               OP.mult)
                nc.vector.scalar_tensor_tensor(q[:], q[:], 0.5,
                                               pg[:, 512:768],
                                               OP.mult, OP.add)
                qs.append(q)
            nns = []
            for q in qs:
                nn = gtp.tile([128, D], f32, tag="nn", bufs=3)
                nc.scalar.activation(nn[:], q[:], AF.Tanh)
                nns.append(nn)
            for nb, trz, nn in zip(nbs, trzs, nns):
                # ho = nn + z*(x-nn) = nn + 0.5*(tz+1)*(x-nn)
                # residual uses x's bf16-hi only: the dropped lo term adds
                # ~1e-3 rel err in quadrature but avoids a WAR hazard on xnl
                dd = gtp.tile([128, D], f32, tag="dd", bufs=2)
                nc.vector.tensor_tensor(dd[:], xnh[:, nb, :], nn[:],
                                        OP.subtract)
                nc.vector.scalar_tensor_tensor(dd[:], trz[:, D:2 * D], 1.0,
                                               dd[:], OP.add, OP.mult)
                ho = gtp.tile([128, D], f32, tag="ho", bufs=2)
                nc.vector.scalar_tensor_tensor(ho[:], dd[:], 0.5, nn[:],
                                               OP.mult, OP.add)
                nc.sync.dma_start(out_d[el, nb * 128:(nb + 1) * 128, :],
                                  ho[:])

        def waves(t, el, mid_hook=None):
            """Interleaved: G/exp(c) [ACT] || conv2(c) [PE] || acc+GRU(c-1)."""
            skipA = "A" in skip
            skipC = "conv2" in skip
            skipG = "gru" in skip
            for c in range(LC):
                if not skipA:
                    g_exp(t, el, c)
                if not skipC:
                    conv2_chunk(t, 0, c)
                    conv2_chunk(t, 1, c)
                if c >= 1:
                    if not skipA:
                        emit_acc(t, c - 1)
                    if not skipG:
                        gru_pair(t, el, 4 * (c - 1))
                        gru_pair(t, el, 4 * (c - 1) + 2)
                if c == 1 and mid_hook is not None:
                    mid_hook()
            if not skipA:
                emit_acc(t, LC - 1)
            if not skipG:
                gru_pair(t, el, 4 * (LC - 1))
                gru_pair(t, el, 4 * (LC - 1) + 2)

        # ---------------- emission schedule ----------------
        els = [e for _ in range(body_reps) for e in range(PER)]
        cur = front(els[0])
        made = []

        def make_hook(el_next):
            def hook():
                made.append(front(el_next))
            return hook

        for idx in range(1, len(els)):
            waves(cur, els[idx - 1], mid_hook=make_hook(els[idx]))
            cur = made[-1]
        waves(cur, els[-1])

    nc.compile()
    return nc


def _host_prep(inputs):
    g = {k: np.asarray(v, np.float32) for k, v in inputs.items()}
    sc1 = g["bn1_g"] / np.sqrt(g["bn1_v"] + EPS)
    sh1 = g["bn1_b"] - g["bn1_m"] * sc1
    sc2 = g["bn2_g"] / np.sqrt(g["bn2_v"] + EPS)
    sh2 = g["bn2_b"] - g["bn2_m"] * sc2

    w1p = g["conv1_w"][:, :, 0] * sc1[:, None]          # (O, I)
    w2p = g["conv2_w"] * sc2[:, None, None]             # (O, I, 17)

    CDT = E4 if CFG["fp8_conv"] else BF
    HDT = E4 if CFG["fp8_hh"] else BF
    IDT = E4 if CFG["fp8_ih"] else BF
    NDT = np.float32 if CFG["f32r_ih"] else BF

    def lhsT_pack(w):   # (O, I) -> (128, kc=I/128, mc=O/128, 128): [p,kc,mc,m]
        o, i = w.shape
        return np.ascontiguousarray(np.transpose(
            w.T.reshape(i // 128, 128, o // 128, 128), (1, 0, 2, 3)))

    w1t = lhsT_pack(w1p).astype(CDT)
    w2t = np.stack([lhsT_pack(w2p[:, :, t]) for t in range(KT)], axis=2)
    w2t = np.ascontiguousarray(np.transpose(w2t, (0, 1, 2, 3, 4)))  # [p,kc,t,mc,m]
    w2t = w2t.astype(CDT)

    def rhs_pack(wt):   # (Kdim, F) -> (128, kc, F)
        k, f = wt.shape
        return np.ascontiguousarray(
            np.transpose(wt.reshape(k // 128, 128, f), (1, 0, 2)))

    wmsgt = rhs_pack(g["w_msg"].T).astype(BF)
    wih = g["w_ih"].T                                    # (512, 768)
    wihct = rhs_pack(wih[0:D]).astype(IDT)
    wihnt = rhs_pack(wih[D:2 * D]).astype(NDT)
    whht = rhs_pack(g["w_hh"].T).astype(HDT)

    wse_t = g["w_se"].T                                  # (256, 3)
    wse_hi = wse_t.astype(BF)
    wse_lo = (wse_t - wse_hi.astype(np.float32)).astype(BF)
    wseth = rhs_pack(wse_hi.astype(np.float32)).astype(BF)
    wsetl = rhs_pack(wse_lo.astype(np.float32)).astype(BF)

    bih, bhh = g["b_ih"], g["b_hh"]
    feed = {
        "w1t": w1t, "w2t": w2t,
        "sh1": np.ascontiguousarray(sh1.reshape(DC, 128).T.astype(np.float32)),
        "sh2": np.ascontiguousarray(sh2.reshape(DC, 128).T.astype(np.float32)),
        "wmsgt": wmsgt, "bmsg": g["b_msg"].reshape(1, D).astype(BF),
        "wseth": wseth, "wsetl": wsetl,
        "bse": g["b_se"].reshape(1, 3).astype(BF),
        "wihct": wihct, "wihnt": wihnt, "whht": whht,
        "brow_rz": (bih[:2 * D] + bhh[:2 * D]).reshape(1, 2 * D).astype(BF),
        "brow_gin": bih[2 * D:].reshape(1, D).astype(BF),
        "brow_ghn": bhh[2 * D:].reshape(1, D).astype(BF),
        "ones128": np.ones((1, 128), BF),
        "ones512": np.ones((1, 512), BF),
        "ones2n": np.ones((2, N), BF),
    }
    return g, feed


def make_in_maps(inputs):
    g, feed = _host_prep(inputs)
    x = g["x"]
    mask = g["mask"]
    use_mask = not bool(np.all(mask == 1.0))
    use_bias = not (np.all(g["b_se"] == 0) and np.all(g["b_msg"] == 0)
                    and np.all(g["b_ih"] == 0) and np.all(g["b_hh"] == 0))
    in_maps = []
    for i in range(NCORE):
        m = dict(feed)
        m["x"] = np.ascontiguousarray(x[i * PER:(i + 1) * PER])
        if use_mask:
            m["maskt"] = np.ascontiguousarray(
                mask[i * PER:(i + 1) * PER].transpose(0, 2, 1)).astype(BF)
        in_maps.append(m)
    return in_maps, use_mask, use_bias


def get_nc(use_mask: bool, use_bias: bool = True):
    key = (use_mask, use_bias, tuple(sorted(CFG.items())))
    if key not in _built:
        _built[key] = _build(use_mask, use_bias)
    return _built[key]


def kernel(**inputs) -> np.ndarray:
    in_maps, use_mask, use_bias = make_in_maps(inputs)
    nc = get_nc(use_mask, use_bias)
    from concourse import bass_utils
    last_err = None
    for attempt in range(3):
        try:
            res = bass_utils.run_bass_kernel_spmd(nc, in_maps,
                                                  core_ids=list(range(NCORE)))
            out = np.concatenate([res.results[i]["out"] for i in range(NCORE)],
                                 axis=0)
            return np.ascontiguousarray(out.astype(np.float32))
        except Exception as e:  # wedged device: reset backend and retry
            last_err = e
            try:
                import jax
                jax.clear_caches()
                jax.extend.backend.clear_backends()
            except Exception:
                pass
            import time as _t
            _t.sleep(5)
    raise last_err
